# revision 6
# baseline (speedup 1.0000x reference)
"""CrossViewAttention Trainium2 Bass kernel (v2).

Math (per batch-group b of NV=8 views, identical to reference):
  kmean[b,j]   = mean_s(x[b,j,s,:]) @ Wk + bk            (linearity of mean)
  scores       = (x @ Wq + bq) . kmean * scale
               = x @ Wqm[b] + bqm[b]                      (fold Wq into kmean)
  w            = softmax_j(scores)
  out[b,i,s]   = sum_j w[b,i,j,h,s] * v[b,j,s,(h,:)],  v = x @ Wv   (+bv folded)
  y            = out @ Wo + (bv @ Wo + bo)                (softmax sums to 1)

Sharding: 8 cores = 4 batch-groups x 2 S-halves. The only cross-core
quantity is the per-(b,view) feature-sum of x over S, exchanged with a
tiny AllReduce ([4,8,512] f32, full 8-core group with one-hot masking so
the SPMD program needs no core-dependent addressing).

v2 layout notes:
 - x / acc transposes run on the DMA xbar (dma_start_transpose, bf16)
   instead of PE transposes + scalar PSUM->SBUF copies.
 - the cross-view mix runs as per-head scalar_tensor_tensor on VectorE
   (per-partition scalar = softmax weight column) with GPSIMD taking the
   j=0 product (acc init) and the top CVA_NGP j's as broadcast-multiply
   products that VectorE then accumulates.
"""

import os
import numpy as np
from contextlib import ExitStack

import concourse.bass as bass
import concourse.bacc as bacc
import concourse.tile as tile
import concourse.mybir as mybir
from concourse.bass_utils import run_bass_kernel_spmd
from concourse.masks import make_identity

# ---- problem constants (hardcoded; kernel.py must be self-contained) ----
B, NV, S, D, H, HD = 4, 8, 2048, 512, 8, 64
NCORES = 8
SPC = int(os.environ.get("CVA_SPC", S // 2))
SH = SPC // 128         # s-blocks of 128
T = D // 128            # 4 contraction tiles
SCALE = HD ** -0.5

F32 = mybir.dt.float32
BF16 = mybir.dt.bfloat16

REPLICA_GROUPS = [list(range(NCORES))]


def _cfg():
    return dict(
        ngp=int(os.environ.get("CVA_NGP", "2")),   # j's on gpsimd as tmp products
        gpinit=int(os.environ.get("CVA_GPINIT", "1")),  # j=0 init on gpsimd
        reps=int(os.environ.get("CVA_REPS", "1")),
    )


def build_kernel(cfg):
    nc = bacc.Bacc(
        "TRN2", target_bir_lowering=False, debug=False, num_devices=NCORES
    )

    x = nc.dram_tensor("x", [NV, SPC, D], F32, kind="ExternalInput").ap()
    Wq = nc.dram_tensor("Wq", [D, D], F32, kind="ExternalInput").ap()
    bq = nc.dram_tensor("bq", [D], F32, kind="ExternalInput").ap()
    Wk = nc.dram_tensor("Wk", [D, D], F32, kind="ExternalInput").ap()
    bk = nc.dram_tensor("bk", [D], F32, kind="ExternalInput").ap()
    Wv = nc.dram_tensor("Wv", [D, D], F32, kind="ExternalInput").ap()
    bv = nc.dram_tensor("bv", [D], F32, kind="ExternalInput").ap()
    Wo = nc.dram_tensor("Wo", [D, D], F32, kind="ExternalInput").ap()
    bo = nc.dram_tensor("bo", [D], F32, kind="ExternalInput").ap()
    mask = nc.dram_tensor("mask", [128, B], F32, kind="ExternalInput").ap()
    y = nc.dram_tensor("y", [NV, SPC, D], F32, kind="ExternalOutput").ap()

    with tile.TileContext(nc) as tc:
        for _rep in range(cfg.get("reps", 1)):
            _body(tc, cfg, x, Wq, bq, Wk, bk, Wv, bv, Wo, bo, mask, y)

    nc.compile()
    return nc


def _body(tc, cfg, x, Wq, bq, Wk, bk, Wv, bv, Wo, bo, mask, y):
    nc = tc.nc
    Exp = mybir.ActivationFunctionType.Exp
    ADD = mybir.AluOpType.add
    MULT = mybir.AluOpType.mult

    ctx = ExitStack()
    with ctx:
        consts = ctx.enter_context(tc.tile_pool(name="consts", bufs=1))

        # ---- constants / weights resident in SBUF ----
        ident_f32 = consts.tile([128, 128], F32)
        make_identity(nc, ident_f32[:, :])
        ones_mm = consts.tile([1, 128], BF16)
        nc.vector.memset(ones_mm[:, :], 1.0)
        # onehots[:, i, :] = column-i one-hot [128, NV] (xsum stationary)
        onehots = consts.tile([128, NV, NV], BF16)
        nc.vector.memset(onehots[:, :, :], 0.0)
        for i in range(NV):
            nc.vector.memset(onehots[:, i, i : i + 1], 1.0)
        one1_f32 = consts.tile([1, 1], F32)
        nc.vector.memset(one1_f32[:, :], 1.0)

        # moving-operand weights [d_in(128), t, d_out(512)]
        wv_sb = consts.tile([128, T, D], BF16)
        wo_sb = consts.tile([128, T, D], BF16)
        nc.gpsimd.dma_start(
            out=wv_sb[:, :, :], in_=Wv.rearrange("(t p) n -> p t n", p=128)
        )
        nc.gpsimd.dma_start(
            out=wo_sb[:, :, :], in_=Wo.rearrange("(t p) n -> p t n", p=128)
        )
        wk_sb = consts.tile([128, T, D], F32)
        nc.sync.dma_start(
            out=wk_sb[:, :, :], in_=Wk.rearrange("(t p) n -> p t n", p=128)
        )
        wq_sb = consts.tile([128, T, D], F32)
        nc.sync.dma_start(
            out=wq_sb[:, :, :], in_=Wq.rearrange("(t p) n -> p t n", p=128)
        )
        bk1 = consts.tile([1, D], F32)
        nc.sync.dma_start(out=bk1[:, :], in_=bk.unsqueeze(0))
        bo1 = consts.tile([1, D], F32)
        nc.sync.dma_start(out=bo1[:, :], in_=bo.unsqueeze(0))
        bqT = consts.tile([128, T], F32)
        nc.sync.dma_start(out=bqT[:, :], in_=bq.rearrange("(t p) -> p t", p=128))
        bvT = consts.tile([128, T], BF16)
        nc.gpsimd.dma_start(out=bvT[:, :], in_=bv.rearrange("(t p) -> p t", p=128))
        mask_sb = consts.tile([128, B], F32)
        nc.sync.dma_start(out=mask_sb[:, :], in_=mask)

        # resident row-major staged activations [s%128, view, s_blk, d]
        stg = consts.tile([128, NV, SH, D], BF16)

        # ---- phase 1: load x + per-view feature sums ----
        ph1 = ExitStack()
        with ph1:
            xs_ps = ph1.enter_context(
                tc.tile_pool(name="xs_ps", bufs=1, space="PSUM")
            )
            xsum_ps = xs_ps.tile([NV, D], F32)
            for i in range(NV):
                nc.gpsimd.dma_start(
                    out=stg[:, i, :, :],
                    in_=x[i].rearrange("(sh p) d -> p sh d", p=128),
                )
                for sh in range(SH):
                    nc.tensor.matmul(
                        xsum_ps[:, :],
                        onehots[:, i, :],
                        stg[:, i, sh, :],
                        start=(i == 0 and sh == 0),
                        stop=(i == NV - 1 and sh == SH - 1),
                    )
            xsum_sb = consts.tile([NV, D], F32)
            nc.scalar.copy(out=xsum_sb[:, :], in_=xsum_ps[:, :])

        # ---- phase 2: exchange partial sums (masked full-group AllReduce) ----
        ph2 = ExitStack()
        with ph2:
            dram = ph2.enter_context(
                tc.tile_pool(name="dram", bufs=1, space="DRAM")
            )
            sb2 = ph2.enter_context(tc.tile_pool(name="sb2", bufs=1))
            xsum4 = sb2.tile([NV, B, D], F32)
            for bb in range(B):
                nc.vector.tensor_scalar(
                    xsum4[:, bb, :],
                    xsum_sb[:, :],
                    mask_sb[0:NV, bb : bb + 1],
                    None,
                    op0=MULT,
                )
            cc_in = dram.tile([B, NV, D], F32)
            cc_out = dram.tile([B, NV, D], F32, addr_space="Shared")
            nc.sync.dma_start(
                out=cc_in[:, :, :].rearrange("b j d -> j b d"),
                in_=xsum4[:, :, :],
            )
            nc.gpsimd.collective_compute(
                "AllReduce",
                ADD,
                replica_groups=REPLICA_GROUPS,
                ins=[cc_in[:, :, :]],
                outs=[cc_out[:, :, :]],
            )
            # pull back all 4 groups, mask-select ours, scale by 1/S * scale
            xsf4 = sb2.tile([128, B, T, NV], F32)
            for bb in range(B):
                for t in range(T):
                    nc.sync.dma_start(
                        out=xsf4[:, bb, t, :],
                        in_=cc_out[bb, :, t * 128 : (t + 1) * 128].rearrange(
                            "j p -> p j"
                        ),
                    )
            xsf = sb2.tile([128, T, NV], F32)
            nc.vector.tensor_scalar(
                xsf[:, :, :],
                xsf4[:, 0, :, :],
                mask_sb[:, 0:1],
                None,
                op0=MULT,
            )
            for bb in range(1, B):
                nc.vector.scalar_tensor_tensor(
                    out=xsf[:, :, :],
                    in0=xsf4[:, bb, :, :],
                    scalar=mask_sb[:, bb : bb + 1],
                    in1=xsf[:, :, :],
                    op0=MULT,
                    op1=ADD,
                )
            nc.vector.tensor_scalar(
                xsf[:, :, :], xsf[:, :, :], SCALE / (2 * SPC), None, op0=MULT
            )

            # ---- phase 3: kmeanT, Wqm, bqm, bo' ----
            km_ps = ph2.enter_context(
                tc.tile_pool(name="km_ps", bufs=1, space="PSUM")
            )
            bk1s = sb2.tile([1, D], F32)
            nc.scalar.mul(bk1s[:, :], bk1[:, :], SCALE)
            ones_j = sb2.tile([1, NV], F32)
            nc.vector.memset(ones_j[:, :], 1.0)
            kmT = sb2.tile([128, T, NV], F32)
            for to in range(T):
                kmT_ps = km_ps.tile([128, NV], F32, tag="kmt", bufs=2)
                nc.tensor.matmul(
                    kmT_ps[:, :],
                    bk1s[:, to * 128 : (to + 1) * 128],
                    ones_j[:, :],
                    start=True,
                    stop=False,
                )
                for t in range(T):
                    nc.tensor.matmul(
                        kmT_ps[:, :],
                        wk_sb[:, t, to * 128 : (to + 1) * 128],
                        xsf[:, t, :],
                        start=False,
                        stop=(t == T - 1),
                    )
                nc.scalar.copy(out=kmT[:, to, :], in_=kmT_ps[:, :])

            # kmH: block-diagonal head-masked copy of kmT.
            # kmH[p, tk, h*NV+j] = kmT[p, tk, j] if head(tk*128+p)==h else 0
            kmH = sb2.tile([128, T, H * NV], F32)
            nc.vector.memset(kmH[:, :, :], 0.0)
            for h in range(H):
                po = (h % 2) * 64
                th = h // 2
                nc.vector.tensor_copy(
                    kmH[po : po + 64, th, h * NV : (h + 1) * NV],
                    kmT[po : po + 64, th, :],
                )

            # wqT via PE transpose of wq_sb
            wqT = sb2.tile([128, T, D], F32)
            tp2 = ph2.enter_context(
                tc.tile_pool(name="tp2", bufs=2, space="PSUM")
            )
            for tr in range(T):
                for tcol in range(T):
                    tp = tp2.tile([128, 128], F32, tag="tpq")
                    nc.tensor.transpose(
                        tp[:, :],
                        wq_sb[:, tr, tcol * 128 : (tcol + 1) * 128],
                        ident_f32[:, :],
                    )
                    nc.scalar.copy(
                        out=wqT[:, tcol, tr * 128 : (tr + 1) * 128], in_=tp[:, :]
                    )

            # wqm[:, td, (h,j)] = sum_hd WqT[hd, td-block] * kmH[hd, (h,j)]
            wqm = consts.tile([128, T, H * NV], BF16)
            bqm = consts.tile([1, H * NV], BF16)
            for td in range(T):
                wqm_ps = km_ps.tile([128, H * NV], F32, tag="wqm", bufs=2)
                for tk in range(T):
                    nc.tensor.matmul(
                        wqm_ps[:, :],
                        wqT[:, tk, td * 128 : (td + 1) * 128],
                        kmH[:, tk, :],
                        start=(tk == 0),
                        stop=(tk == T - 1),
                    )
                nc.scalar.copy(out=wqm[:, td, :], in_=wqm_ps[:, :])
            bqm_ps = km_ps.tile([1, H * NV], F32, tag="bias_ps", bufs=2)
            for tk in range(T):
                nc.tensor.matmul(
                    bqm_ps[:, :],
                    bqT[:, tk : tk + 1],
                    kmH[:, tk, :],
                    start=(tk == 0),
                    stop=(tk == T - 1),
                )
            nc.scalar.copy(out=bqm[:, :], in_=bqm_ps[:, :])

            # bo' = bv @ Wo + bo
            bop = consts.tile([1, D], BF16)
            bop_ps = km_ps.tile([1, D], F32, tag="bias_ps", bufs=2)
            nc.tensor.matmul(
                bop_ps[:, :], one1_f32[:, :], bo1[:, :], start=True, stop=False
            )
            for t in range(T):
                nc.tensor.matmul(
                    bop_ps[:, :],
                    bvT[:, t : t + 1],
                    wo_sb[:, t, :],
                    start=False,
                    stop=(t == T - 1),
                )
            nc.scalar.copy(out=bop[:, :], in_=bop_ps[:, :])

        # ---- phase 4: main loop over s-blocks ----
        sc_ps = ctx.enter_context(tc.tile_pool(name="sc_ps", bufs=2, space="PSUM"))
        v_ps = ctx.enter_context(tc.tile_pool(name="v_ps", bufs=2, space="PSUM"))
        y_ps = ctx.enter_context(tc.tile_pool(name="y_ps", bufs=2, space="PSUM"))
        xtb_pool = ctx.enter_context(tc.tile_pool(name="xtb", bufs=2))
        vw_pool = ctx.enter_context(tc.tile_pool(name="vw", bufs=10))
        sm_pool = ctx.enter_context(tc.tile_pool(name="sm", bufs=3))
        acc_pool = ctx.enter_context(tc.tile_pool(name="acc", bufs=10))
        yo_pool = ctx.enter_context(tc.tile_pool(name="yo", bufs=3))

        ngp = cfg["ngp"]
        gp_js = set(range(NV - ngp, NV)) if ngp > 0 else set()
        gpinit = bool(cfg["gpinit"])

        for blk in range(SH):
            rs = slice(blk * 128, (blk + 1) * 128)
            # transpose this block of x for all views on the DMA xbar:
            # xTb[d%128, t, i, s']
            xTb = xtb_pool.tile([128, T, NV, 128], BF16, tag="xtb")
            for i in range(NV):
                for t in range(T):
                    nc.sync.dma_start_transpose(
                        out=xTb[:, t, i, :],
                        in_=stg[:, i, blk, t * 128 : (t + 1) * 128],
                    )
            v_sb = []
            wt_sb = []
            for i in range(NV):
                scp = sc_ps.tile([128, H * NV], F32, tag="scp")
                nc.tensor.matmul(
                    scp[:, :],
                    ones_mm[:, :],
                    bqm[:, :],
                    start=True,
                    stop=False,
                )
                vp = v_ps.tile([128, D], F32, tag="vp")
                for t in range(T):
                    nc.tensor.matmul(
                        vp[:, :],
                        xTb[:, t, i, :],
                        wv_sb[:, t, :],
                        start=(t == 0),
                        stop=(t == T - 1),
                    )
                    nc.tensor.matmul(
                        scp[:, :],
                        xTb[:, t, i, :],
                        wqm[:, t, :],
                        start=False,
                        stop=(t == T - 1),
                    )
                vt = vw_pool.tile([128, H, HD], BF16, tag="v")
                nc.scalar.copy(
                    out=vt[:, :, :].rearrange("p h d -> p (h d)"), in_=vp[:, :]
                )
                v_sb.append(vt)

                e_sb = sm_pool.tile([128, H, NV], BF16, tag="e")
                nc.scalar.activation(
                    e_sb[:, :, :].rearrange("p h j -> p (h j)"), scp[:, :], Exp
                )
                z = sm_pool.tile([128, H], F32, tag="z")
                nc.vector.tensor_reduce(
                    z[:, :],
                    e_sb[:, :, :],
                    axis=mybir.AxisListType.X,
                    op=ADD,
                )
                rz = sm_pool.tile([128, H], F32, tag="rz")
                nc.vector.reciprocal(rz[:, :], z[:, :])
                wt = vw_pool.tile([128, H, NV], BF16, tag="w")
                nc.vector.tensor_tensor(
                    out=wt[:, :, :],
                    in0=e_sb[:, :, :],
                    in1=rz[:, :].unsqueeze(2).broadcast_to([128, H, NV]),
                    op=MULT,
                )
                wt_sb.append(wt)

            # cross-view mix: acc[i] = sum_j w[i,(h,j)] * v[j,(h,:)]
            accs = []
            for i in range(NV):
                acc = acc_pool.tile([128, H, HD], BF16, tag="acc")
                accs.append(acc)
                wt = wt_sb[i]
                # j=0: product into acc (gpsimd broadcast-mult or DVE per-head TS)
                if gpinit:
                    nc.gpsimd.tensor_tensor(
                        out=acc[:, :, :],
                        in0=v_sb[0][:, :, :],
                        in1=wt[:, :, 0:1].broadcast_to([128, H, HD]),
                        op=MULT,
                    )
                else:
                    for h in range(H):
                        nc.vector.tensor_scalar(
                            acc[:, h, :],
                            v_sb[0][:, h, :],
                            wt[:, h, 0:1],
                            None,
                            op0=MULT,
                        )
                # middle j's: fused MAC on DVE per head
                for j in range(1, NV):
                    if j in gp_js:
                        continue
                    for h in range(H):
                        nc.vector.scalar_tensor_tensor(
                            out=acc[:, h, :],
                            in0=v_sb[j][:, h, :],
                            scalar=wt[:, h, j : j + 1],
                            in1=acc[:, h, :],
                            op0=MULT,
                            op1=ADD,
                        )
                # top j's: gpsimd broadcast products, DVE accumulates
                tmps = []
                for j in sorted(gp_js):
                    tmp = acc_pool.tile([128, H, HD], BF16, tag=f"gp{j}")
                    nc.gpsimd.tensor_tensor(
                        out=tmp[:, :, :],
                        in0=v_sb[j][:, :, :],
                        in1=wt[:, :, j : j + 1].broadcast_to([128, H, HD]),
                        op=MULT,
                    )
                    tmps.append(tmp)
                for tmp in tmps:
                    nc.vector.tensor_tensor(
                        out=acc[:, :, :],
                        in0=acc[:, :, :],
                        in1=tmp[:, :, :],
                        op=ADD,
                    )

            # y projection: transpose acc on the DMA xbar, then matmul
            for i in range(NV):
                accT = yo_pool.tile([128, T, 128], BF16, tag="accT")
                flat = accs[i][:, :, :].rearrange("p h d -> p (h d)")
                for c in range(T):
                    nc.sync.dma_start_transpose(
                        out=accT[:, c, :],
                        in_=flat[:, c * 128 : (c + 1) * 128],
                    )
                yp = y_ps.tile([128, D], F32, tag="yp")
                nc.tensor.matmul(
                    yp[:, :],
                    ones_mm[:, :],
                    bop[:, :],
                    start=True,
                    stop=False,
                )
                for c in range(T):
                    nc.tensor.matmul(
                        yp[:, :],
                        accT[:, c, :],
                        wo_sb[:, c, :],
                        start=False,
                        stop=(c == T - 1),
                    )
                y_sb = yo_pool.tile([128, D], F32, tag="ysb")
                nc.scalar.copy(out=y_sb[:, :], in_=yp[:, :])
                nc.sync.dma_start(out=y[i, rs, :], in_=y_sb[:, :])


_BUILD_CACHE = {}
LAST_RESULT = None


def _get_nc(cfg):
    key = tuple(sorted(cfg.items()))
    if key not in _BUILD_CACHE:
        _BUILD_CACHE[key] = build_kernel(cfg)
    return _BUILD_CACHE[key]


def kernel(**inputs):
    global LAST_RESULT
    cfg = _cfg()
    nc = _get_nc(cfg)

    x = np.asarray(inputs["x"], dtype=np.float32)
    weights = {
        k: np.ascontiguousarray(np.asarray(inputs[k], dtype=np.float32))
        for k in ["Wq", "bq", "Wk", "bk", "Wv", "bv", "Wo", "bo"]
    }

    in_maps = []
    for c in range(NCORES):
        b, half = c // 2, c % 2
        xs = np.ascontiguousarray(
            x[b * NV : (b + 1) * NV, half * SPC : (half + 1) * SPC, :]
        )
        m = np.zeros((128, B), dtype=np.float32)
        m[:, b] = 1.0
        im = {"x": xs, "mask": m}
        im.update(weights)
        in_maps.append(im)

    res = run_bass_kernel_spmd(
        nc,
        in_maps,
        core_ids=list(range(NCORES)),
        trace=bool(int(os.environ.get("CVA_TRACE", "0"))),
    )
    LAST_RESULT = res

    out = np.empty((B * NV, S, D), dtype=np.float32)
    for c in range(NCORES):
        b, half = c // 2, c % 2
        out[b * NV : (b + 1) * NV, half * SPC : (half + 1) * SPC, :] = res.results[
            c
        ]["y"]
    return out


# revision 11
# speedup vs baseline: 1.5737x; 1.5737x over previous
"""CrossViewAttention Trainium2 Bass kernel (v2).

Math (per batch-group b of NV=8 views, identical to reference):
  kmean[b,j]   = mean_s(x[b,j,s,:]) @ Wk + bk            (linearity of mean)
  scores       = (x @ Wq + bq) . kmean * scale
               = x @ Wqm[b] + bqm[b]                      (fold Wq into kmean)
  w            = softmax_j(scores)
  out[b,i,s]   = sum_j w[b,i,j,h,s] * v[b,j,s,(h,:)],  v = x @ Wv   (+bv folded)
  y            = out @ Wo + (bv @ Wo + bo)                (softmax sums to 1)

Sharding: 8 cores = 4 batch-groups x 2 S-halves. The only cross-core
quantity is the per-(b,view) feature-sum of x over S, exchanged with a
tiny AllReduce ([4,8,512] f32, full 8-core group with one-hot masking so
the SPMD program needs no core-dependent addressing).

v2 layout notes:
 - x / acc transposes run on the DMA xbar (dma_start_transpose, bf16)
   instead of PE transposes + scalar PSUM->SBUF copies.
 - the cross-view mix runs as per-head scalar_tensor_tensor on VectorE
   (per-partition scalar = softmax weight column) with GPSIMD taking the
   j=0 product (acc init) and the top CVA_NGP j's as broadcast-multiply
   products that VectorE then accumulates.
"""

import os
import numpy as np
from contextlib import ExitStack

import concourse.bass as bass
import concourse.bacc as bacc
import concourse.tile as tile
import concourse.mybir as mybir
from concourse.bass_utils import run_bass_kernel_spmd
from concourse.masks import make_identity

# ---- problem constants (hardcoded; kernel.py must be self-contained) ----
B, NV, S, D, H, HD = 4, 8, 2048, 512, 8, 64
NCORES = 8
SPC = int(os.environ.get("CVA_SPC", S // 2))
SH = SPC // 128         # s-blocks of 128
T = D // 128            # 4 contraction tiles
SCALE = HD ** -0.5

F32 = mybir.dt.float32
BF16 = mybir.dt.bfloat16

REPLICA_GROUPS = [list(range(NCORES))]


def _cfg():
    return dict(
        ngp=int(os.environ.get("CVA_NGP", "3")),   # j's on gpsimd as tmp products
        gpinit=int(os.environ.get("CVA_GPINIT", "1")),  # j=0 init on gpsimd
        reps=int(os.environ.get("CVA_REPS", "1")),
    )


def build_kernel(cfg):
    nc = bacc.Bacc(
        "TRN2", target_bir_lowering=False, debug=False, num_devices=NCORES
    )

    x = nc.dram_tensor("x", [NV, SPC, D], F32, kind="ExternalInput").ap()
    Wq = nc.dram_tensor("Wq", [D, D], F32, kind="ExternalInput").ap()
    bq = nc.dram_tensor("bq", [D], F32, kind="ExternalInput").ap()
    Wk = nc.dram_tensor("Wk", [D, D], F32, kind="ExternalInput").ap()
    bk = nc.dram_tensor("bk", [D], F32, kind="ExternalInput").ap()
    Wv = nc.dram_tensor("Wv", [D, D], F32, kind="ExternalInput").ap()
    bv = nc.dram_tensor("bv", [D], F32, kind="ExternalInput").ap()
    Wo = nc.dram_tensor("Wo", [D, D], F32, kind="ExternalInput").ap()
    bo = nc.dram_tensor("bo", [D], F32, kind="ExternalInput").ap()
    mask = nc.dram_tensor("mask", [128, B], F32, kind="ExternalInput").ap()
    y = nc.dram_tensor("y", [NV, SPC, D], F32, kind="ExternalOutput").ap()

    with tile.TileContext(nc) as tc:
        for _rep in range(cfg.get("reps", 1)):
            _body(tc, cfg, x, Wq, bq, Wk, bk, Wv, bv, Wo, bo, mask, y)

    nc.compile()
    return nc


def _body(tc, cfg, x, Wq, bq, Wk, bk, Wv, bv, Wo, bo, mask, y):
    nc = tc.nc
    Exp = mybir.ActivationFunctionType.Exp
    ADD = mybir.AluOpType.add
    MULT = mybir.AluOpType.mult

    ctx = ExitStack()
    with ctx:
        consts = ctx.enter_context(tc.tile_pool(name="consts", bufs=1))

        # ---- constants / weights resident in SBUF ----
        ident_f32 = consts.tile([128, 128], F32)
        make_identity(nc, ident_f32[:, :])
        ones_mm = consts.tile([1, 128], BF16)
        nc.vector.memset(ones_mm[:, :], 1.0)
        # onehots[:, i, :] = column-i one-hot [128, NV] (xsum stationary)
        onehots = consts.tile([128, NV, NV], BF16)
        nc.vector.memset(onehots[:, :, :], 0.0)
        for i in range(NV):
            nc.vector.memset(onehots[:, i, i : i + 1], 1.0)
        one1_f32 = consts.tile([1, 1], F32)
        nc.vector.memset(one1_f32[:, :], 1.0)

        # moving-operand weights [d_in(128), t, d_out(512)]
        wv_sb = consts.tile([128, T, D], BF16)
        wo_sb = consts.tile([128, T, D], BF16)
        nc.gpsimd.dma_start(
            out=wv_sb[:, :, :], in_=Wv.rearrange("(t p) n -> p t n", p=128)
        )
        nc.gpsimd.dma_start(
            out=wo_sb[:, :, :], in_=Wo.rearrange("(t p) n -> p t n", p=128)
        )
        wk_sb = consts.tile([128, T, D], F32)
        nc.sync.dma_start(
            out=wk_sb[:, :, :], in_=Wk.rearrange("(t p) n -> p t n", p=128)
        )
        wq_sb = consts.tile([128, T, D], F32)
        nc.sync.dma_start(
            out=wq_sb[:, :, :], in_=Wq.rearrange("(t p) n -> p t n", p=128)
        )
        bk1 = consts.tile([1, D], F32)
        nc.sync.dma_start(out=bk1[:, :], in_=bk.unsqueeze(0))
        bo1 = consts.tile([1, D], F32)
        nc.sync.dma_start(out=bo1[:, :], in_=bo.unsqueeze(0))
        bqT = consts.tile([128, T], F32)
        nc.sync.dma_start(out=bqT[:, :], in_=bq.rearrange("(t p) -> p t", p=128))
        bvT = consts.tile([128, T], BF16)
        nc.gpsimd.dma_start(out=bvT[:, :], in_=bv.rearrange("(t p) -> p t", p=128))
        mask_sb = consts.tile([128, B], F32)
        nc.sync.dma_start(out=mask_sb[:, :], in_=mask)

        # resident row-major staged activations [s%128, view, s_blk, d]
        stg = consts.tile([128, NV, SH, D], BF16)

        # ---- phase 1: load x + per-view feature sums ----
        ph1 = ExitStack()
        with ph1:
            xs_ps = ph1.enter_context(
                tc.tile_pool(name="xs_ps", bufs=1, space="PSUM")
            )
            xsum_ps = xs_ps.tile([NV, D], F32)
            for i in range(NV):
                nc.gpsimd.dma_start(
                    out=stg[:, i, :, :],
                    in_=x[i].rearrange("(sh p) d -> p sh d", p=128),
                )
                for sh in range(SH):
                    nc.tensor.matmul(
                        xsum_ps[:, :],
                        onehots[:, i, :],
                        stg[:, i, sh, :],
                        start=(i == 0 and sh == 0),
                        stop=(i == NV - 1 and sh == SH - 1),
                    )
            xsum_sb = consts.tile([NV, D], F32)
            nc.scalar.copy(out=xsum_sb[:, :], in_=xsum_ps[:, :])

        # ---- phase 2: exchange partial sums (masked full-group AllReduce) ----
        ph2 = ExitStack()
        with ph2:
            dram = ph2.enter_context(
                tc.tile_pool(name="dram", bufs=1, space="DRAM")
            )
            sb2 = ph2.enter_context(tc.tile_pool(name="sb2", bufs=1))
            xsum4 = sb2.tile([NV, B, D], F32)
            for bb in range(B):
                nc.vector.tensor_scalar(
                    xsum4[:, bb, :],
                    xsum_sb[:, :],
                    mask_sb[0:NV, bb : bb + 1],
                    None,
                    op0=MULT,
                )
            cc_in = dram.tile([B, NV, D], F32)
            cc_out = dram.tile([B, NV, D], F32, addr_space="Shared")
            nc.sync.dma_start(
                out=cc_in[:, :, :].rearrange("b j d -> j b d"),
                in_=xsum4[:, :, :],
            )
            nc.gpsimd.collective_compute(
                "AllReduce",
                ADD,
                replica_groups=REPLICA_GROUPS,
                ins=[cc_in[:, :, :]],
                outs=[cc_out[:, :, :]],
            )
            # pull back all 4 groups, mask-select ours, scale by 1/S * scale
            xsf4 = sb2.tile([128, B, T, NV], F32)
            for bb in range(B):
                for t in range(T):
                    nc.sync.dma_start(
                        out=xsf4[:, bb, t, :],
                        in_=cc_out[bb, :, t * 128 : (t + 1) * 128].rearrange(
                            "j p -> p j"
                        ),
                    )
            xsf = sb2.tile([128, T, NV], F32)
            nc.vector.tensor_scalar(
                xsf[:, :, :],
                xsf4[:, 0, :, :],
                mask_sb[:, 0:1],
                None,
                op0=MULT,
            )
            for bb in range(1, B):
                nc.vector.scalar_tensor_tensor(
                    out=xsf[:, :, :],
                    in0=xsf4[:, bb, :, :],
                    scalar=mask_sb[:, bb : bb + 1],
                    in1=xsf[:, :, :],
                    op0=MULT,
                    op1=ADD,
                )
            nc.vector.tensor_scalar(
                xsf[:, :, :], xsf[:, :, :], SCALE / (2 * SPC), None, op0=MULT
            )

            # ---- phase 3: kmeanT, Wqm, bqm, bo' ----
            km_ps = ph2.enter_context(
                tc.tile_pool(name="km_ps", bufs=1, space="PSUM")
            )
            bk1s = sb2.tile([1, D], F32)
            nc.scalar.mul(bk1s[:, :], bk1[:, :], SCALE)
            ones_j = sb2.tile([1, NV], F32)
            nc.vector.memset(ones_j[:, :], 1.0)
            kmT = sb2.tile([128, T, NV], F32)
            for to in range(T):
                kmT_ps = km_ps.tile([128, NV], F32, tag="kmt", bufs=2)
                nc.tensor.matmul(
                    kmT_ps[:, :],
                    bk1s[:, to * 128 : (to + 1) * 128],
                    ones_j[:, :],
                    start=True,
                    stop=False,
                )
                for t in range(T):
                    nc.tensor.matmul(
                        kmT_ps[:, :],
                        wk_sb[:, t, to * 128 : (to + 1) * 128],
                        xsf[:, t, :],
                        start=False,
                        stop=(t == T - 1),
                    )
                nc.scalar.copy(out=kmT[:, to, :], in_=kmT_ps[:, :])

            # kmH: block-diagonal head-masked copy of kmT.
            # kmH[p, tk, h*NV+j] = kmT[p, tk, j] if head(tk*128+p)==h else 0
            kmH = sb2.tile([128, T, H * NV], F32)
            nc.vector.memset(kmH[:, :, :], 0.0)
            for h in range(H):
                po = (h % 2) * 64
                th = h // 2
                nc.vector.tensor_copy(
                    kmH[po : po + 64, th, h * NV : (h + 1) * NV],
                    kmT[po : po + 64, th, :],
                )

            # wqT via PE transpose of wq_sb
            wqT = sb2.tile([128, T, D], F32)
            tp2 = ph2.enter_context(
                tc.tile_pool(name="tp2", bufs=2, space="PSUM")
            )
            for tr in range(T):
                for tcol in range(T):
                    tp = tp2.tile([128, 128], F32, tag="tpq")
                    nc.tensor.transpose(
                        tp[:, :],
                        wq_sb[:, tr, tcol * 128 : (tcol + 1) * 128],
                        ident_f32[:, :],
                    )
                    nc.scalar.copy(
                        out=wqT[:, tcol, tr * 128 : (tr + 1) * 128], in_=tp[:, :]
                    )

            # wqm[:, td, (h,j)] = sum_hd WqT[hd, td-block] * kmH[hd, (h,j)]
            wqm = consts.tile([128, T, H * NV], BF16)
            bqm = consts.tile([1, H * NV], BF16)
            for td in range(T):
                wqm_ps = km_ps.tile([128, H * NV], F32, tag="wqm", bufs=2)
                for tk in range(T):
                    nc.tensor.matmul(
                        wqm_ps[:, :],
                        wqT[:, tk, td * 128 : (td + 1) * 128],
                        kmH[:, tk, :],
                        start=(tk == 0),
                        stop=(tk == T - 1),
                    )
                nc.scalar.copy(out=wqm[:, td, :], in_=wqm_ps[:, :])
            bqm_ps = km_ps.tile([1, H * NV], F32, tag="bias_ps", bufs=2)
            for tk in range(T):
                nc.tensor.matmul(
                    bqm_ps[:, :],
                    bqT[:, tk : tk + 1],
                    kmH[:, tk, :],
                    start=(tk == 0),
                    stop=(tk == T - 1),
                )
            nc.scalar.copy(out=bqm[:, :], in_=bqm_ps[:, :])

            # bo' = bv @ Wo + bo
            bop = consts.tile([1, D], BF16)
            bop_ps = km_ps.tile([1, D], F32, tag="bias_ps", bufs=2)
            nc.tensor.matmul(
                bop_ps[:, :], one1_f32[:, :], bo1[:, :], start=True, stop=False
            )
            for t in range(T):
                nc.tensor.matmul(
                    bop_ps[:, :],
                    bvT[:, t : t + 1],
                    wo_sb[:, t, :],
                    start=False,
                    stop=(t == T - 1),
                )
            nc.scalar.copy(out=bop[:, :], in_=bop_ps[:, :])

        # ---- phase 4: main loop over s-blocks ----
        sc_ps = ctx.enter_context(tc.tile_pool(name="sc_ps", bufs=1, space="PSUM"))
        v_ps = ctx.enter_context(tc.tile_pool(name="v_ps", bufs=2, space="PSUM"))
        y_ps = ctx.enter_context(tc.tile_pool(name="y_ps", bufs=2, space="PSUM"))
        xtp_ps = ctx.enter_context(
            tc.tile_pool(name="xtp_ps", bufs=2, space="PSUM")
        )
        tp_ps2 = ctx.enter_context(
            tc.tile_pool(name="tp_ps2", bufs=1, space="PSUM")
        )
        xtb_pool = ctx.enter_context(tc.tile_pool(name="xtb", bufs=2))
        vw_pool = ctx.enter_context(tc.tile_pool(name="vw", bufs=10))
        sm_pool = ctx.enter_context(tc.tile_pool(name="sm", bufs=3))
        acc_pool = ctx.enter_context(tc.tile_pool(name="acc", bufs=10))
        yo_pool = ctx.enter_context(tc.tile_pool(name="yo", bufs=3))

        ident_bf16 = consts.tile([128, 128], BF16)
        make_identity(nc, ident_bf16[:, :])

        ngp = cfg["ngp"]
        gp_js = set(range(NV - ngp, NV)) if ngp > 0 else set()
        gpinit = bool(cfg["gpinit"])

        for blk in range(SH):
            rs = slice(blk * 128, (blk + 1) * 128)
            # transpose this block of x for all views: xTb[d%128, t, i, s']
            xTb = xtb_pool.tile([128, T, NV, 128], BF16, tag="xtb")
            for i in range(NV):
                for t in range(T):
                    tp = xtp_ps.tile([128, 128], BF16, tag="xtp")
                    nc.tensor.transpose(
                        tp[:, :],
                        stg[:, i, blk, t * 128 : (t + 1) * 128],
                        ident_bf16[:, :],
                    )
                    nc.scalar.copy(out=xTb[:, t, i, :], in_=tp[:, :])
            v_sb = []
            wt_sb = []
            for i in range(NV):
                scp = sc_ps.tile([128, H * NV], F32, tag="scp")
                nc.tensor.matmul(
                    scp[:, :],
                    ones_mm[:, :],
                    bqm[:, :],
                    start=True,
                    stop=False,
                )
                vp = v_ps.tile([128, D], F32, tag="vp")
                for t in range(T):
                    nc.tensor.matmul(
                        vp[:, :],
                        xTb[:, t, i, :],
                        wv_sb[:, t, :],
                        start=(t == 0),
                        stop=(t == T - 1),
                    )
                    nc.tensor.matmul(
                        scp[:, :],
                        xTb[:, t, i, :],
                        wqm[:, t, :],
                        start=False,
                        stop=(t == T - 1),
                    )
                vt = vw_pool.tile([128, H, HD], BF16, tag="v")
                nc.scalar.copy(
                    out=vt[:, :, :].rearrange("p h d -> p (h d)"), in_=vp[:, :]
                )
                v_sb.append(vt)

                e_sb = sm_pool.tile([128, H, NV], BF16, tag="e")
                nc.scalar.activation(
                    e_sb[:, :, :].rearrange("p h j -> p (h j)"), scp[:, :], Exp
                )
                z = sm_pool.tile([128, H], F32, tag="z")
                nc.vector.tensor_reduce(
                    z[:, :],
                    e_sb[:, :, :],
                    axis=mybir.AxisListType.X,
                    op=ADD,
                )
                rz = sm_pool.tile([128, H], F32, tag="rz")
                nc.vector.reciprocal(rz[:, :], z[:, :])
                wt = vw_pool.tile([128, H, NV], BF16, tag="w")
                nc.vector.tensor_tensor(
                    out=wt[:, :, :],
                    in0=e_sb[:, :, :],
                    in1=rz[:, :].unsqueeze(2).broadcast_to([128, H, NV]),
                    op=MULT,
                )
                wt_sb.append(wt)

            # cross-view mix: acc[i] = sum_j w[i,(h,j)] * v[j,(h,:)]
            # gpsimd computes gp_js products (plus the j=0 init when gpinit);
            # vector computes the remaining products and all accumulate-adds.
            accs = []
            for i in range(NV):
                acc = acc_pool.tile([128, H, HD], BF16, tag="acc")
                accs.append(acc)
                wt = wt_sb[i]
                flat_acc = acc[:, :, :].rearrange("p h d -> p (h d)")
                # j=0: product straight into acc
                if gpinit:
                    nc.gpsimd.tensor_tensor(
                        out=acc[:, :, :],
                        in0=v_sb[0][:, :, :],
                        in1=wt[:, :, 0:1].broadcast_to([128, H, HD]),
                        op=MULT,
                    )
                else:
                    nc.vector.tensor_tensor(
                        out=acc[:, :, :],
                        in0=v_sb[0][:, :, :],
                        in1=wt[:, :, 0:1].broadcast_to([128, H, HD]),
                        op=MULT,
                    )
                # remaining j's: broadcast product into tmp (engine by split),
                # then flat 2x-mode accumulate on vector
                for j in range(1, NV):
                    eng = nc.gpsimd if j in gp_js else nc.vector
                    tmp = acc_pool.tile([128, H, HD], BF16, tag=f"mt{j % 2}")
                    eng.tensor_tensor(
                        out=tmp[:, :, :],
                        in0=v_sb[j][:, :, :],
                        in1=wt[:, :, j : j + 1].broadcast_to([128, H, HD]),
                        op=MULT,
                    )
                    nc.vector.tensor_tensor(
                        out=flat_acc,
                        in0=flat_acc,
                        in1=tmp[:, :, :].rearrange("p h d -> p (h d)"),
                        op=ADD,
                    )

            # y projection: transpose acc on PE, then matmul
            for i in range(NV):
                accT = yo_pool.tile([128, T, 128], BF16, tag="accT")
                flat = accs[i][:, :, :].rearrange("p h d -> p (h d)")
                for c in range(T):
                    tp = tp_ps2.tile([128, 128], BF16, tag="tpy")
                    nc.tensor.transpose(
                        tp[:, :],
                        flat[:, c * 128 : (c + 1) * 128],
                        ident_bf16[:, :],
                    )
                    nc.scalar.copy(out=accT[:, c, :], in_=tp[:, :])
                yp = y_ps.tile([128, D], F32, tag="yp")
                nc.tensor.matmul(
                    yp[:, :],
                    ones_mm[:, :],
                    bop[:, :],
                    start=True,
                    stop=False,
                )
                for c in range(T):
                    nc.tensor.matmul(
                        yp[:, :],
                        accT[:, c, :],
                        wo_sb[:, c, :],
                        start=False,
                        stop=(c == T - 1),
                    )
                y_sb = yo_pool.tile([128, D], F32, tag="ysb")
                nc.scalar.copy(out=y_sb[:, :], in_=yp[:, :])
                nc.sync.dma_start(out=y[i, rs, :], in_=y_sb[:, :])


_BUILD_CACHE = {}
LAST_RESULT = None


def _get_nc(cfg):
    key = tuple(sorted(cfg.items()))
    if key not in _BUILD_CACHE:
        _BUILD_CACHE[key] = build_kernel(cfg)
    return _BUILD_CACHE[key]


def kernel(**inputs):
    global LAST_RESULT
    cfg = _cfg()
    nc = _get_nc(cfg)

    x = np.asarray(inputs["x"], dtype=np.float32)
    weights = {
        k: np.ascontiguousarray(np.asarray(inputs[k], dtype=np.float32))
        for k in ["Wq", "bq", "Wk", "bk", "Wv", "bv", "Wo", "bo"]
    }

    in_maps = []
    for c in range(NCORES):
        b, half = c // 2, c % 2
        xs = np.ascontiguousarray(
            x[b * NV : (b + 1) * NV, half * SPC : (half + 1) * SPC, :]
        )
        m = np.zeros((128, B), dtype=np.float32)
        m[:, b] = 1.0
        im = {"x": xs, "mask": m}
        im.update(weights)
        in_maps.append(im)

    res = run_bass_kernel_spmd(
        nc,
        in_maps,
        core_ids=list(range(NCORES)),
        trace=bool(int(os.environ.get("CVA_TRACE", "0"))),
    )
    LAST_RESULT = res

    out = np.empty((B * NV, S, D), dtype=np.float32)
    for c in range(NCORES):
        b, half = c // 2, c % 2
        out[b * NV : (b + 1) * NV, half * SPC : (half + 1) * SPC, :] = res.results[
            c
        ]["y"]
    return out
